# revision 1
# baseline (speedup 1.0000x reference)
"""KNRM kernel for 8 TRN2 NeuronCores (data-parallel over batch).

mmT dataflow: match matrix computed transposed ([d-tokens on partitions,
q-cols on free]); doc-sums accumulated on the TensorEngine via ones-column
(E_k slices of one Z matrix) matmuls into [21, 512] PSUM tiles; tail
(log1p via Ln-with-scale-AP, w-fold via Copy-with-scale-AP, q-sum reduce,
gpsimd partition reduce, sigmoid via exp+reciprocal).

Per half-super (16 batch rows): 40 single-offset indirect gathers (bf16,
64-padded rows), 40 PE transposes batched 8-per-PSUM-bank with Act-engine
copies (all operands kept at base partition 0 -- base 64 crashes the HW),
32 mm matmuls (bf16, N=32), 2-group bf16 RBF chain (restart depth 10),
42 S-accumulation matmuls, tail. One activation-function table
(natural_log_exp_and_others) serves Exp/Square/Ln/Copy so no
LoadActFuncSet churn (see _patched_act_tables).
"""
import math
import numpy as np

import concourse.bass as bass
import concourse.bacc as bacc
import concourse.mybir as mybir
import concourse.tile as tile
from concourse import bass_isa
from concourse import bass_utils

F32 = mybir.dt.float32
BF16 = mybir.dt.bfloat16
I32 = mybir.dt.int32
AF = mybir.ActivationFunctionType
ALU = mybir.AluOpType

N_CORES = 8
B, LQ, LD = 4096, 30, 128
V, D = 400000, 50
DP = 64                    # table padded to 64 dims (256B rows)
KN = 21
BC = B // N_CORES          # 512 b per core
NHALF = BC // 16           # 32 half-supers of 16 b
NGRP = 32                  # (b,pred)-groups per half: 16 b x 2 pred
GP = 32                    # group pitch: 30 q-cols + 2 pad (bank aligned)
QC = NGRP * GP             # 1024 q-cols per half (incl pad)
CHUNK = 512                # S-matmul chunk = one PSUM bank at f32
NCHUNK = QC // CHUNK       # 2

MUS = []
for i in range(KN):
    mu = 1.0 / (KN - 1) + 2.0 * i / (KN - 1) - 1.0
    MUS.append(min(mu, 1.0))
GROUPS = [(0, MUS[0]), (10, MUS[10])]  # 2 chain groups, j=0..9
GLEN = 10
DCONST = np.zeros(KN, np.float64)
for k0, mu_f in GROUPS:
    for j in range(GLEN):
        DCONST[k0 + j] = math.exp(-10.0 * j * mu_f - j * j / 2.0)
DCONST[20] = 1.0  # exact-match count slot

# d-tile slot for group g within a half-super, chosen so the d-tile's
# partition half after pair-transposes (t%2) equals the q-tile half (gi%2).
TLOC = []
for _g in range(NGRP):
    _gi, _s4 = _g // 4, _g % 4
    _h = _gi % 2
    TLOC.append(2 * (4 * ((_gi - _h) // 2) + _s4) + _h)
assert sorted(TLOC) == list(range(NGRP))


def _patched_act_tables(orig_fn):
    """Strip our activation funcs from every table except the one that
    holds all of them, so the table-load pass resolves Exp/Square/Ln/Copy
    to a single set (indexes preserved; only set contents change)."""
    import functools

    @functools.cache
    def wrapper(arch):
        ours = {AF.Exp, AF.Square, AF.Ln, AF.Copy}
        out = {}
        for name, s in orig_fn(arch).items():
            if name == "natural_log_exp_and_others":
                out[name] = s
            else:
                out[name] = s - ours
        return out

    return wrapper


def _build_nc(nhalf=NHALF, stage=5):
    if not getattr(bacc, "_knrm_act_patch", False):
        bacc.get_activation_tables = _patched_act_tables(
            bacc.get_activation_tables)
        bacc._knrm_act_patch = True
    nc = bacc.Bacc("TRN2", target_bir_lowering=False, debug=False,
                   num_devices=N_CORES)
    emb = nc.dram_tensor("embn", [V, DP], BF16, kind="ExternalInput")
    qidx = nc.dram_tensor("qidx", [128, BC // 2], I32, kind="ExternalInput")
    didx = nc.dram_tensor("didx", [128, BC * 2], I32, kind="ExternalInput")
    ident = nc.dram_tensor("ident", [128, 128], BF16, kind="ExternalInput")
    zmat = nc.dram_tensor("zmat", [128, 2 * KN - 1], BF16, kind="ExternalInput")
    dwp = nc.dram_tensor("dwp", [KN, 1], F32, kind="ExternalInput")
    wp = nc.dram_tensor("wp", [KN, 1], F32, kind="ExternalInput")
    y = nc.dram_tensor("y", [BC, 1], F32, kind="ExternalOutput")

    with tile.TileContext(nc) as tc, nc.allow_low_precision("bf16 rbf chain"):
        with (
            tc.tile_pool(name="const", bufs=1) as cpool,
            tc.tile_pool(name="idx", bufs=1) as ipool,
            tc.tile_pool(name="gath", bufs=3) as gpool,
            tc.tile_pool(name="tp", bufs=2, space="PSUM") as tppool,
            tc.tile_pool(name="mm", bufs=2, space="PSUM") as mmpool,
            tc.tile_pool(name="S", bufs=1, space="PSUM") as spool,
            tc.tile_pool(name="eT", bufs=2) as epool,
            tc.tile_pool(name="v", bufs=2) as vpool,
            tc.tile_pool(name="sq", bufs=2) as sqpool,
            tc.tile_pool(name="chain", bufs=2) as chpool,
            tc.tile_pool(name="tail", bufs=2) as tpool,
            tc.tile_pool(name="out", bufs=1) as opool,
        ):
            identt = cpool.tile([128, 128], BF16, tag="ident")
            nc.sync.dma_start(out=identt[:], in_=ident.ap())
            zt = cpool.tile([128, 2 * KN - 1], BF16, tag="zmat")
            nc.sync.dma_start(out=zt[:], in_=zmat.ap())
            dwt = cpool.tile([KN, 1], F32, tag="dw")
            nc.sync.dma_start(out=dwt[:], in_=dwp.ap())
            wt = cpool.tile([KN, 1], F32, tag="w")
            nc.sync.dma_start(out=wt[:], in_=wp.ap())
            bias_aps = {}
            for (k0g, mu_fg) in GROUPS:
                bt = cpool.tile([128, 1], F32, tag=f"bias{k0g}",
                                name=f"bias{k0g}")
                nc.gpsimd.memset(bt[:], -float(mu_fg))
                bias_aps[k0g] = bt
            qit = ipool.tile([128, BC // 2], I32, tag="qi")
            nc.sync.dma_start(out=qit[:], in_=qidx.ap())
            dit = ipool.tile([128, BC * 2], I32, tag="di")
            nc.sync.dma_start(out=dit[:], in_=didx.ap())
            ysb = opool.tile([1, BC], F32, tag="y")
            lrall = opool.tile([KN, 2 * BC], F32, tag="LrAll")
            nc.gpsimd.memset(lrall[:], 0.0)

            for s2 in range(nhalf):
                # ---- gathers: [128,1] offsets (HW-validated) ----
                def gather1(tag, cols, col):
                    gt = gpool.tile([128, DP], BF16, tag=tag, name=tag)
                    nc.gpsimd.indirect_dma_start(
                        out=gt[:], out_offset=None, in_=emb.ap(),
                        in_offset=bass.IndirectOffsetOnAxis(
                            ap=cols[:, col:col + 1], axis=0))
                    return gt

                qgs = [gather1(f"qg{u}", qit, 8 * s2 + u) for u in range(8)]
                dgs = [gather1(f"dg{m % 16}", dit, 32 * s2 + m)
                       for m in range(32)]

                if stage == 1:
                    nc.vector.tensor_copy(out=ysb[:, 0:64],
                                          in_=qgs[0][0:1, :])
                    continue
                # ---- single-tile transposes [128,DP]->[DP,128], 8 per
                # PSUM bank, all data at base partition 0 ----
                tpq = tppool.tile([DP, 1024], BF16, tag="tp")
                for u in range(8):
                    nc.tensor.transpose(
                        out=tpq[:, 128 * u:128 * u + 128],
                        in_=qgs[u][:], identity=identt[:])
                eqQs = []
                for a in range(2):
                    eqQ = epool.tile([DP, 512], BF16, tag=f"q{a}",
                                     name=f"eqQ{a}")
                    nc.scalar.copy(out=eqQ[:], in_=tpq[:, 512 * a:512 * a + 512])
                    eqQs.append(eqQ)
                edLs = []
                for i in range(4):
                    tpd = tppool.tile([DP, 1024], BF16, tag="tp")
                    for u in range(8):
                        nc.tensor.transpose(
                            out=tpd[:, 128 * u:128 * u + 128],
                            in_=dgs[8 * i + u][:], identity=identt[:])
                    for a in range(2):
                        edL = epool.tile([DP, 512], BF16, tag=f"e{2 * i + a}",
                                         name=f"edL{2 * i + a}")
                        nc.scalar.copy(out=edL[:],
                                       in_=tpd[:, 512 * a:512 * a + 512])
                        edLs.append(edL)

                if stage == 2:
                    nc.vector.tensor_copy(out=ysb[:, 0:64],
                                          in_=eqQs[0][0:1, 0:64])
                    continue
                # ---- mm matmuls: 32 groups, out [128 d, 30 q] each ----
                mmT = mmpool.tile([128, QC], F32, tag="mm")
                for g in range(NGRP):
                    gi = g // 4
                    lhsT = edLs[g // 4][0:D, 128 * (g % 4):128 * (g % 4) + 128]
                    rhs = eqQs[gi // 4][0:D,
                                        128 * (gi % 4) + LQ * (g % 4):
                                        128 * (gi % 4) + LQ * (g % 4) + GP]
                    nc.tensor.matmul(out=mmT[:, GP * g:GP * g + GP],
                                     lhsT=lhsT, rhs=rhs,
                                     start=True, stop=True)

                if stage == 3:
                    nc.vector.tensor_copy(out=ysb[:, 0:512],
                                          in_=mmT[0:1, 0:512])
                    continue
                # ---- RBF chain + S accumulation ----
                vt = vpool.tile([128, QC], BF16, tag="V")
                nc.scalar.activation(vt[:], mmT[:], AF.Exp, scale=10.0)
                Sps = [spool.tile([KN, CHUNK], F32, tag=f"S{c}",
                                  name=f"Sps{c}")
                       for c in range(NCHUNK)]

                def smm(k, kt):
                    for c in range(NCHUNK):
                        nc.tensor.matmul(
                            out=Sps[c][:],
                            lhsT=zt[:, KN - 1 - k:2 * KN - 1 - k],
                            rhs=kt[:, CHUNK * c:CHUNK * c + CHUNK],
                            start=(k == 0), stop=(k == KN - 1))

                for (k0, mu_f) in GROUPS:
                    sqt = sqpool.tile([128, QC], F32, tag="sq")
                    nc.scalar.activation(sqt[:], mmT[:], AF.Square,
                                         bias=bias_aps[k0][:])
                    kt = chpool.tile([128, QC], BF16, tag=f"K{k0 % 3}")
                    nc.scalar.activation(kt[:], sqt[:], AF.Exp, scale=-50.0)
                    smm(k0, kt)
                    for j in range(1, GLEN):
                        kt2 = chpool.tile([128, QC], BF16,
                                          tag=f"K{(k0 + j) % 3}")
                        nc.vector.tensor_tensor(out=kt2[:], in0=kt[:],
                                                in1=vt[:], op=ALU.mult)
                        smm(k0 + j, kt2)
                        kt = kt2
                ind = chpool.tile([128, QC], BF16, tag="ind")
                nc.vector.tensor_scalar(out=ind[:], in0=vt[:],
                                        scalar1=10000.0, scalar2=None,
                                        op0=ALU.is_ge)
                smm(KN - 1, ind)

                if stage == 4:
                    nc.vector.tensor_copy(out=ysb[:, 0:512],
                                          in_=Sps[0][0:1, :])
                    continue
                # ---- tail: log1p, w-fold, q-sum (k-sum deferred) ----
                for c in range(NCHUNK):
                    L = tpool.tile([KN, CHUNK], F32, tag=f"L{c}")
                    nc.scalar.activation(L[:], Sps[c][:], AF.Ln,
                                         bias=1.0, scale=dwt[:])
                    Lw = tpool.tile([KN, CHUNK], F32, tag=f"Lw{c}")
                    nc.scalar.activation(Lw[:], L[:], AF.Copy,
                                         scale=wt[:])
                    nc.vector.tensor_reduce(
                        out=lrall[:, 32 * s2 + 16 * c:32 * s2 + 16 * c + 16],
                        in_=Lw[:].rearrange("p (g q) -> p g q",
                                            q=GP)[:, :, 0:LQ],
                        axis=mybir.AxisListType.X, op=ALU.add)

            # ---- deferred tail: one k-sum + sigmoid over all halves ----
            kred = opool.tile([KN, 2 * BC], F32, tag="kr")
            nc.gpsimd.partition_all_reduce(
                out_ap=kred[:], in_ap=lrall[:], channels=KN,
                reduce_op=bass_isa.ReduceOp.add)
            kv = kred[0:1, :].rearrange("p (i two) -> p two i", two=2)
            df = opool.tile([1, BC], F32, tag="df")
            nc.vector.tensor_tensor(out=df[:], in0=kv[:, 0, :],
                                    in1=kv[:, 1, :], op=ALU.subtract)
            ex = opool.tile([1, BC], F32, tag="ex")
            nc.scalar.activation(ex[:], df[:], AF.Exp, scale=-1.0)
            e1 = opool.tile([1, BC], F32, tag="e1")
            nc.vector.tensor_scalar(out=e1[:], in0=ex[:], scalar1=1.0,
                                    scalar2=None, op0=ALU.add)
            nc.vector.reciprocal(out=ysb[:], in_=e1[:])
            nc.sync.dma_start(out=y.ap().rearrange("b o -> o b"), in_=ysb[:])
    nc.compile()
    return nc


_NC_CACHE = None


def _get_nc():
    global _NC_CACHE
    if _NC_CACHE is None:
        _NC_CACHE = _build_nc()
    return _NC_CACHE


def make_inputs(q1, d1, q2, d2, emb, mlp_w):
    """Host-side prep shared by kernel() and tests."""
    import ml_dtypes
    emb = np.asarray(emb, dtype=np.float32)
    nrm = np.sqrt((emb * emb).sum(axis=1, keepdims=True))
    embn = np.zeros((V, DP), ml_dtypes.bfloat16)
    embn[:, :D] = emb / np.maximum(nrm, np.float32(1e-12))
    ident = np.eye(128, dtype=ml_dtypes.bfloat16)
    zmatv = np.zeros((128, 2 * KN - 1), ml_dtypes.bfloat16)
    zmatv[:, KN - 1] = 1.0
    dwv = DCONST.astype(np.float32).reshape(KN, 1)
    wv = np.asarray(mlp_w, np.float32).reshape(KN, 1)

    in_maps = []
    for c in range(N_CORES):
        b0 = c * BC
        q1c, q2c = q1[b0:b0 + BC], q2[b0:b0 + BC]
        d1c, d2c = d1[b0:b0 + BC], d2[b0:b0 + BC]
        qidx = np.zeros((BC // 2, 128), np.int32)
        qidx[:, 0:30] = q1c[0::2]
        qidx[:, 30:60] = q2c[0::2]
        qidx[:, 60:90] = q1c[1::2]
        qidx[:, 90:120] = q2c[1::2]
        didx = np.empty((BC * 2, 128), np.int32)
        didx[0::2] = d1c
        didx[1::2] = d2c
        in_maps.append({
            "embn": embn, "qidx": np.ascontiguousarray(qidx.T),
            "didx": np.ascontiguousarray(didx.T), "ident": ident,
            "zmat": zmatv, "dwp": dwv, "wp": wv,
        })
    return in_maps


class _Runner:
    """Cached jitted SPMD executor (avoids per-call retrace/recompile)."""

    def __init__(self, nc):
        import jax
        from jax.sharding import Mesh, PartitionSpec, NamedSharding
        from jax.experimental.shard_map import shard_map
        from concourse import bass2jax

        bass2jax.install_neuronx_cc_hook()
        self.jax = jax
        self.nc = nc
        pname = nc.partition_id_tensor.name if nc.partition_id_tensor else None
        self.in_names, self.out_names, self.out_avals = [], [], []
        for alloc in nc.m.functions[0].allocations:
            if not isinstance(alloc, mybir.MemoryLocationSet):
                continue
            name = alloc.memorylocations[0].name
            if alloc.kind == "ExternalInput":
                if name != pname and (nc.dbg_addr is None
                                      or name != nc.dbg_addr.name):
                    self.in_names.append(name)
            elif alloc.kind == "ExternalOutput":
                self.out_names.append(name)
                self.out_avals.append(jax.core.ShapedArray(
                    tuple(alloc.tensor_shape), mybir.dt.np(alloc.dtype)))
        n_params = len(self.in_names)
        all_in = list(self.in_names) + list(self.out_names)
        if nc.dbg_addr is not None:
            all_in.append(nc.dbg_addr.name)
        if pname is not None:
            all_in.append(pname)

        def _body(*args):
            operands = list(args)
            if nc.dbg_addr is not None:
                operands.append(jax.numpy.zeros((1, 2), np.uint32))
            if pname is not None:
                operands.append(bass2jax.partition_id_tensor())
            outs = bass2jax._bass_exec_p.bind(
                *operands,
                out_avals=tuple(self.out_avals),
                in_names=tuple(all_in),
                out_names=tuple(self.out_names),
                lowering_input_output_aliases=(),
                sim_require_finite=True,
                sim_require_nnan=True,
                nc=nc,
            )
            return tuple(outs)

        devices = jax.devices()[:N_CORES]
        self.mesh = Mesh(np.asarray(devices), ("core",))
        n_outs = len(self.out_names)
        in_specs = (PartitionSpec("core"),) * (n_params + n_outs)
        out_specs = (PartitionSpec("core"),) * n_outs
        self.sharded = jax.jit(
            shard_map(_body, mesh=self.mesh, in_specs=in_specs,
                      out_specs=out_specs, check_rep=False),
            donate_argnums=tuple(range(n_params, n_params + n_outs)),
            keep_unused=True)
        self.shard = NamedSharding(self.mesh, PartitionSpec("core"))
        self._const_cache = {}

    def put_const(self, key, build_fn):
        hit = self._const_cache.get(key[0])
        if hit is not None and hit[0] is key[1]:
            return hit[1]
        arr = self.jax.device_put(build_fn(), self.shard)
        self.jax.block_until_ready(arr)
        self._const_cache[key[0]] = (key[1], arr)
        return arr

    def run(self, in_map_arrays):
        jax = self.jax
        args = [in_map_arrays[name] for name in self.in_names]
        zeros = [jax.device_put(
            np.zeros((N_CORES * a.shape[0], *a.shape[1:]), a.dtype),
            self.shard) for a in self.out_avals]
        outs = self.sharded(*args, *zeros)
        outs = [np.asarray(o) for o in outs]
        return {name: outs[i].reshape(N_CORES, *self.out_avals[i].shape)
                for i, name in enumerate(self.out_names)}


_RUNNER = None


def _get_runner():
    global _RUNNER
    if _RUNNER is None:
        _RUNNER = _Runner(_get_nc())
    return _RUNNER


def kernel(q1, d1, q2, d2, emb, mlp_w, mlp_b):
    q1 = np.asarray(q1); d1 = np.asarray(d1)
    q2 = np.asarray(q2); d2 = np.asarray(d2)
    last_err = None
    for _attempt in range(3):
        try:
            r = _get_runner()
            in_maps = make_inputs(q1, d1, q2, d2, emb, mlp_w)
            arrays = {}
            for name in r.in_names:
                if name == "embn":
                    arrays[name] = r.put_const(
                        ("embn", emb),
                        lambda: np.concatenate(
                            [in_maps[c]["embn"] for c in range(N_CORES)],
                            axis=0))
                else:
                    cat = np.concatenate(
                        [np.asarray(in_maps[c][name])
                         for c in range(N_CORES)], axis=0)
                    arrays[name] = r.jax.device_put(cat, r.shard)
            outs = r.run(arrays)
            y = outs["y"].reshape(B, 1)
            return y.astype(np.float32)
        except Exception as e:  # transient axon/device failures
            last_err = e
            global _RUNNER
            _RUNNER = None
    raise last_err



# revision 2
# speedup vs baseline: 36.2859x; 36.2859x over previous
"""KNRM kernel for 8 TRN2 NeuronCores (data-parallel over batch).

mmT dataflow: match matrix computed transposed ([d-tokens on partitions,
q-cols on free]); doc-sums accumulated on the TensorEngine via ones-column
(E_k slices of one Z matrix) matmuls into [21, 512] PSUM tiles; tail
(log1p via Ln-with-scale-AP, w-fold via Copy-with-scale-AP, q-sum reduce,
k-sum via a ones-column PE matmul, sigmoid via exp+reciprocal).

Per half-super (16 batch rows): 40 single-offset indirect gathers (bf16,
64-padded rows), 40 PE transposes batched 8-per-PSUM-bank with Act-engine
copies (all operands kept at base partition 0 -- base 64 crashes the HW),
32 mm matmuls (bf16, N=32), 2-group bf16 RBF chain (restart depth 10),
42 S-accumulation matmuls, tail. One activation-function table
(natural_log_exp_and_others) serves Exp/Square/Ln/Copy so no
LoadActFuncSet churn (see _patched_act_tables).

The indirect embedding gathers are HBM random-row-latency bound
(~10.7 ns/row across 16 SDMA engines) and dominate the device time
(~1.76 ms of ~1.85 ms/call); the k-sum runs on the TensorEngine (ones
matmul) instead of gpsimd partition_all_reduce so the Pool engine's
gather stream is never stalled by the compute tail, and the lrall
accumulator needs no memset (every column is written each iteration).

`_build_nc(reps=N)` unrolls the whole computation N times inside one
NEFF; test.py uses reps=1 vs reps>1 wall-clock slope to measure the true
per-call device execution time under the ~80 ms axon per-call dispatch
floor.
"""
import math
import numpy as np

import concourse.bass as bass
import concourse.bacc as bacc
import concourse.mybir as mybir
import concourse.tile as tile
from concourse import bass_utils  # noqa: F401 (kept for API parity)

F32 = mybir.dt.float32
BF16 = mybir.dt.bfloat16
I32 = mybir.dt.int32
AF = mybir.ActivationFunctionType
ALU = mybir.AluOpType

N_CORES = 8
B, LQ, LD = 4096, 30, 128
V, D = 400000, 50
DP = 64                    # table padded to 64 dims (256B rows)
KN = 21
BC = B // N_CORES          # 512 b per core
NHALF = BC // 16           # 32 half-supers of 16 b
NGRP = 32                  # (b,pred)-groups per half: 16 b x 2 pred
GP = 32                    # group pitch: 30 q-cols + 2 pad (bank aligned)
QC = NGRP * GP             # 1024 q-cols per half (incl pad)
CHUNK = 512                # S-matmul chunk = one PSUM bank at f32
NCHUNK = QC // CHUNK       # 2

MUS = []
for i in range(KN):
    mu = 1.0 / (KN - 1) + 2.0 * i / (KN - 1) - 1.0
    MUS.append(min(mu, 1.0))
GROUPS = [(0, MUS[0]), (10, MUS[10])]  # 2 chain groups, j=0..9
GLEN = 10
DCONST = np.zeros(KN, np.float64)
for k0, mu_f in GROUPS:
    for j in range(GLEN):
        DCONST[k0 + j] = math.exp(-10.0 * j * mu_f - j * j / 2.0)
DCONST[20] = 1.0  # exact-match count slot


def _patched_act_tables(orig_fn):
    """Strip our activation funcs from every table except the one that
    holds all of them, so the table-load pass resolves Exp/Square/Ln/Copy
    to a single set (indexes preserved; only set contents change)."""
    import functools

    @functools.cache
    def wrapper(arch):
        ours = {AF.Exp, AF.Square, AF.Ln, AF.Copy}
        out = {}
        for name, s in orig_fn(arch).items():
            if name == "natural_log_exp_and_others":
                out[name] = s
            else:
                out[name] = s - ours
        return out

    return wrapper


def _build_nc(nhalf=NHALF, reps=1):
    if not getattr(bacc, "_knrm_act_patch", False):
        bacc.get_activation_tables = _patched_act_tables(
            bacc.get_activation_tables)
        bacc._knrm_act_patch = True
    nc = bacc.Bacc("TRN2", target_bir_lowering=False, debug=False,
                   num_devices=N_CORES)
    emb = nc.dram_tensor("embn", [V, DP], BF16, kind="ExternalInput")
    qidx = nc.dram_tensor("qidx", [128, BC // 2], I32, kind="ExternalInput")
    didx = nc.dram_tensor("didx", [128, BC * 2], I32, kind="ExternalInput")
    ident = nc.dram_tensor("ident", [128, 128], BF16, kind="ExternalInput")
    zmat = nc.dram_tensor("zmat", [128, 2 * KN - 1], BF16, kind="ExternalInput")
    dwp = nc.dram_tensor("dwp", [KN, 1], F32, kind="ExternalInput")
    wp = nc.dram_tensor("wp", [KN, 1], F32, kind="ExternalInput")
    y = nc.dram_tensor("y", [BC, 1], F32, kind="ExternalOutput")

    with tile.TileContext(nc) as tc, nc.allow_low_precision("bf16 rbf chain"):
        with (
            tc.tile_pool(name="const", bufs=1) as cpool,
            tc.tile_pool(name="idx", bufs=1) as ipool,
            tc.tile_pool(name="gath", bufs=3) as gpool,
            tc.tile_pool(name="tp", bufs=1, space="PSUM") as tppool,
            tc.tile_pool(name="mm", bufs=2, space="PSUM") as mmpool,
            tc.tile_pool(name="S", bufs=1, space="PSUM") as spool,
            tc.tile_pool(name="eT", bufs=2) as epool,
            tc.tile_pool(name="v", bufs=2) as vpool,
            tc.tile_pool(name="sq", bufs=2) as sqpool,
            tc.tile_pool(name="chain", bufs=2) as chpool,
            tc.tile_pool(name="tail", bufs=2) as tpool,
            tc.tile_pool(name="out", bufs=1) as opool,
        ):
            identt = cpool.tile([128, 128], BF16, tag="ident")
            nc.sync.dma_start(out=identt[:], in_=ident.ap())
            zt = cpool.tile([128, 2 * KN - 1], BF16, tag="zmat")
            nc.sync.dma_start(out=zt[:], in_=zmat.ap())
            dwt = cpool.tile([KN, 1], F32, tag="dw")
            nc.sync.dma_start(out=dwt[:], in_=dwp.ap())
            wt = cpool.tile([KN, 1], F32, tag="w")
            nc.sync.dma_start(out=wt[:], in_=wp.ap())
            bias_aps = {}
            for (k0g, mu_fg) in GROUPS:
                bt = cpool.tile([128, 1], F32, tag=f"bias{k0g}",
                                name=f"bias{k0g}")
                nc.gpsimd.memset(bt[:], -float(mu_fg))
                bias_aps[k0g] = bt
            onest = cpool.tile([KN, 1], F32, tag="ones")
            nc.gpsimd.memset(onest[:], 1.0)
            qit = ipool.tile([128, BC // 2], I32, tag="qi")
            nc.sync.dma_start(out=qit[:], in_=qidx.ap())
            dit = ipool.tile([128, BC * 2], I32, tag="di")
            nc.sync.dma_start(out=dit[:], in_=didx.ap())

            for _rep in range(reps):
                ysb = opool.tile([1, BC], F32, tag="y")
                lrall = opool.tile([KN, 2 * BC], F32, tag="LrAll")

                for s2 in range(nhalf):
                    # ---- gathers: [128,1] offsets (HW-validated) ----
                    def gather1(tag, cols, col):
                        gt = gpool.tile([128, DP], BF16, tag=tag, name=tag)
                        nc.gpsimd.indirect_dma_start(
                            out=gt[:], out_offset=None, in_=emb.ap(),
                            in_offset=bass.IndirectOffsetOnAxis(
                                ap=cols[:, col:col + 1], axis=0))
                        return gt

                    qgs = [gather1(f"qg{u}", qit, 8 * s2 + u)
                           for u in range(8)]
                    dgs = [gather1(f"dg{m % 16}", dit, 32 * s2 + m)
                           for m in range(32)]

                    # ---- single-tile transposes [128,DP]->[DP,128], 8 per
                    # PSUM bank, all data at base partition 0 ----
                    tpq = tppool.tile([DP, 1024], BF16, tag="tp")
                    for u in range(8):
                        nc.tensor.transpose(
                            out=tpq[:, 128 * u:128 * u + 128],
                            in_=qgs[u][:], identity=identt[:])
                    eqQs = []
                    for a in range(2):
                        eqQ = epool.tile([DP, 512], BF16, tag=f"q{a}",
                                         name=f"eqQ{a}")
                        nc.scalar.copy(out=eqQ[:],
                                       in_=tpq[:, 512 * a:512 * a + 512])
                        eqQs.append(eqQ)
                    edLs = []
                    for i in range(4):
                        tpd = tppool.tile([DP, 1024], BF16, tag="tp")
                        for u in range(8):
                            nc.tensor.transpose(
                                out=tpd[:, 128 * u:128 * u + 128],
                                in_=dgs[8 * i + u][:], identity=identt[:])
                        for a in range(2):
                            edL = epool.tile([DP, 512], BF16,
                                             tag=f"e{2 * i + a}",
                                             name=f"edL{2 * i + a}")
                            nc.scalar.copy(out=edL[:],
                                           in_=tpd[:, 512 * a:512 * a + 512])
                            edLs.append(edL)

                    # ---- mm matmuls: 32 groups, out [128 d, 30 q] each ----
                    mmT = mmpool.tile([128, QC], F32, tag="mm")
                    for g in range(NGRP):
                        gi = g // 4
                        lhsT = edLs[g // 4][0:D,
                                            128 * (g % 4):128 * (g % 4) + 128]
                        rhs = eqQs[gi // 4][0:D,
                                            128 * (gi % 4) + LQ * (g % 4):
                                            128 * (gi % 4) + LQ * (g % 4) + GP]
                        nc.tensor.matmul(out=mmT[:, GP * g:GP * g + GP],
                                         lhsT=lhsT, rhs=rhs,
                                         start=True, stop=True)

                    # ---- RBF chain + S accumulation ----
                    vt = vpool.tile([128, QC], BF16, tag="V")
                    nc.scalar.activation(vt[:], mmT[:], AF.Exp, scale=10.0)
                    Sps = [spool.tile([KN, CHUNK], F32, tag=f"S{c}",
                                      name=f"Sps{c}")
                           for c in range(NCHUNK)]

                    def smm(k, kt):
                        for c in range(NCHUNK):
                            nc.tensor.matmul(
                                out=Sps[c][:],
                                lhsT=zt[:, KN - 1 - k:2 * KN - 1 - k],
                                rhs=kt[:, CHUNK * c:CHUNK * c + CHUNK],
                                start=(k == 0), stop=(k == KN - 1))

                    for (k0, mu_f) in GROUPS:
                        sqt = sqpool.tile([128, QC], F32, tag="sq")
                        nc.scalar.activation(sqt[:], mmT[:], AF.Square,
                                             bias=bias_aps[k0][:])
                        kt = chpool.tile([128, QC], BF16, tag=f"K{k0 % 3}")
                        nc.scalar.activation(kt[:], sqt[:], AF.Exp,
                                             scale=-50.0)
                        smm(k0, kt)
                        for j in range(1, GLEN):
                            kt2 = chpool.tile([128, QC], BF16,
                                              tag=f"K{(k0 + j) % 3}")
                            nc.vector.tensor_tensor(out=kt2[:], in0=kt[:],
                                                    in1=vt[:], op=ALU.mult)
                            smm(k0 + j, kt2)
                            kt = kt2
                    ind = chpool.tile([128, QC], BF16, tag="ind")
                    nc.vector.tensor_scalar(out=ind[:], in0=vt[:],
                                            scalar1=10000.0, scalar2=None,
                                            op0=ALU.is_ge)
                    smm(KN - 1, ind)

                    # ---- tail: log1p, w-fold, q-sum (k-sum deferred) ----
                    for c in range(NCHUNK):
                        L = tpool.tile([KN, CHUNK], F32, tag=f"L{c}")
                        nc.scalar.activation(L[:], Sps[c][:], AF.Ln,
                                             bias=1.0, scale=dwt[:])
                        Lw = tpool.tile([KN, CHUNK], F32, tag=f"Lw{c}")
                        nc.scalar.activation(Lw[:], L[:], AF.Copy,
                                             scale=wt[:])
                        nc.vector.tensor_reduce(
                            out=lrall[:, 32 * s2 + 16 * c:32 * s2 + 16 * c + 16],
                            in_=Lw[:].rearrange("p (g q) -> p g q",
                                                q=GP)[:, :, 0:LQ],
                            axis=mybir.AxisListType.X, op=ALU.add)

                # ---- deferred tail: k-sum on the PE (ones matmul; keeps the
                # Pool engine free for gathers) + sigmoid ----
                kred = opool.tile([1, 2 * BC], F32, tag="kr")
                for c in range(2):
                    kredp = spool.tile([1, BC], F32, tag="krp")
                    nc.tensor.matmul(out=kredp[:], lhsT=onest[:],
                                     rhs=lrall[:, BC * c:BC * (c + 1)],
                                     start=True, stop=True)
                    nc.scalar.copy(out=kred[:, BC * c:BC * (c + 1)],
                                   in_=kredp[:])
                kv = kred[0:1, :].rearrange("p (i two) -> p two i", two=2)
                df = opool.tile([1, BC], F32, tag="df")
                nc.vector.tensor_tensor(out=df[:], in0=kv[:, 0, :],
                                        in1=kv[:, 1, :], op=ALU.subtract)
                ex = opool.tile([1, BC], F32, tag="ex")
                nc.scalar.activation(ex[:], df[:], AF.Exp, scale=-1.0)
                e1 = opool.tile([1, BC], F32, tag="e1")
                nc.vector.tensor_scalar(out=e1[:], in0=ex[:], scalar1=1.0,
                                        scalar2=None, op0=ALU.add)
                nc.vector.reciprocal(out=ysb[:], in_=e1[:])
                nc.sync.dma_start(out=y.ap().rearrange("b o -> o b"),
                                  in_=ysb[:])
    nc.compile()
    return nc


_NC_CACHE = None


def _get_nc():
    global _NC_CACHE
    if _NC_CACHE is None:
        _NC_CACHE = _build_nc()
    return _NC_CACHE


def make_inputs(q1, d1, q2, d2, emb, mlp_w):
    """Host-side prep shared by kernel() and tests."""
    import ml_dtypes
    emb = np.asarray(emb, dtype=np.float32)
    nrm = np.sqrt((emb * emb).sum(axis=1, keepdims=True))
    embn = np.zeros((V, DP), ml_dtypes.bfloat16)
    embn[:, :D] = emb / np.maximum(nrm, np.float32(1e-12))
    ident = np.eye(128, dtype=ml_dtypes.bfloat16)
    zmatv = np.zeros((128, 2 * KN - 1), ml_dtypes.bfloat16)
    zmatv[:, KN - 1] = 1.0
    dwv = DCONST.astype(np.float32).reshape(KN, 1)
    wv = np.asarray(mlp_w, np.float32).reshape(KN, 1)

    in_maps = []
    for c in range(N_CORES):
        b0 = c * BC
        q1c, q2c = q1[b0:b0 + BC], q2[b0:b0 + BC]
        d1c, d2c = d1[b0:b0 + BC], d2[b0:b0 + BC]
        qidx = np.zeros((BC // 2, 128), np.int32)
        qidx[:, 0:30] = q1c[0::2]
        qidx[:, 30:60] = q2c[0::2]
        qidx[:, 60:90] = q1c[1::2]
        qidx[:, 90:120] = q2c[1::2]
        didx = np.empty((BC * 2, 128), np.int32)
        didx[0::2] = d1c
        didx[1::2] = d2c
        in_maps.append({
            "embn": embn, "qidx": np.ascontiguousarray(qidx.T),
            "didx": np.ascontiguousarray(didx.T), "ident": ident,
            "zmat": zmatv, "dwp": dwv, "wp": wv,
        })
    return in_maps


class _Runner:
    """Cached jitted SPMD executor (avoids per-call retrace/recompile)."""

    def __init__(self, nc):
        import jax
        from jax.sharding import Mesh, PartitionSpec, NamedSharding
        from jax.experimental.shard_map import shard_map
        from concourse import bass2jax

        bass2jax.install_neuronx_cc_hook()
        self.jax = jax
        self.nc = nc
        pname = nc.partition_id_tensor.name if nc.partition_id_tensor else None
        self.in_names, self.out_names, self.out_avals = [], [], []
        for alloc in nc.m.functions[0].allocations:
            if not isinstance(alloc, mybir.MemoryLocationSet):
                continue
            name = alloc.memorylocations[0].name
            if alloc.kind == "ExternalInput":
                if name != pname and (nc.dbg_addr is None
                                      or name != nc.dbg_addr.name):
                    self.in_names.append(name)
            elif alloc.kind == "ExternalOutput":
                self.out_names.append(name)
                self.out_avals.append(jax.core.ShapedArray(
                    tuple(alloc.tensor_shape), mybir.dt.np(alloc.dtype)))
        n_params = len(self.in_names)
        all_in = list(self.in_names) + list(self.out_names)
        if nc.dbg_addr is not None:
            all_in.append(nc.dbg_addr.name)
        if pname is not None:
            all_in.append(pname)

        def _body(*args):
            operands = list(args)
            if nc.dbg_addr is not None:
                operands.append(jax.numpy.zeros((1, 2), np.uint32))
            if pname is not None:
                operands.append(bass2jax.partition_id_tensor())
            outs = bass2jax._bass_exec_p.bind(
                *operands,
                out_avals=tuple(self.out_avals),
                in_names=tuple(all_in),
                out_names=tuple(self.out_names),
                lowering_input_output_aliases=(),
                sim_require_finite=True,
                sim_require_nnan=True,
                nc=nc,
            )
            return tuple(outs)

        devices = jax.devices()[:N_CORES]
        self.mesh = Mesh(np.asarray(devices), ("core",))
        n_outs = len(self.out_names)
        in_specs = (PartitionSpec("core"),) * (n_params + n_outs)
        out_specs = (PartitionSpec("core"),) * n_outs
        self.sharded = jax.jit(
            shard_map(_body, mesh=self.mesh, in_specs=in_specs,
                      out_specs=out_specs, check_rep=False),
            donate_argnums=tuple(range(n_params, n_params + n_outs)),
            keep_unused=True)
        self.shard = NamedSharding(self.mesh, PartitionSpec("core"))
        self._const_cache = {}

    def put_const(self, key, build_fn):
        hit = self._const_cache.get(key[0])
        if hit is not None and hit[0] is key[1]:
            return hit[1]
        arr = self.jax.device_put(build_fn(), self.shard)
        self.jax.block_until_ready(arr)
        self._const_cache[key[0]] = (key[1], arr)
        return arr

    def run(self, in_map_arrays):
        jax = self.jax
        args = [in_map_arrays[name] for name in self.in_names]
        zeros = [jax.device_put(
            np.zeros((N_CORES * a.shape[0], *a.shape[1:]), a.dtype),
            self.shard) for a in self.out_avals]
        outs = self.sharded(*args, *zeros)
        outs = [np.asarray(o) for o in outs]
        return {name: outs[i].reshape(N_CORES, *self.out_avals[i].shape)
                for i, name in enumerate(self.out_names)}


_RUNNER = None


def _get_runner():
    global _RUNNER
    if _RUNNER is None:
        _RUNNER = _Runner(_get_nc())
    return _RUNNER


def kernel(q1, d1, q2, d2, emb, mlp_w, mlp_b):
    q1 = np.asarray(q1); d1 = np.asarray(d1)
    q2 = np.asarray(q2); d2 = np.asarray(d2)
    last_err = None
    for _attempt in range(3):
        try:
            r = _get_runner()
            in_maps = make_inputs(q1, d1, q2, d2, emb, mlp_w)
            arrays = {}
            for name in r.in_names:
                if name == "embn":
                    arrays[name] = r.put_const(
                        ("embn", emb),
                        lambda: np.concatenate(
                            [in_maps[c]["embn"] for c in range(N_CORES)],
                            axis=0))
                else:
                    cat = np.concatenate(
                        [np.asarray(in_maps[c][name])
                         for c in range(N_CORES)], axis=0)
                    arrays[name] = r.jax.device_put(cat, r.shard)
            outs = r.run(arrays)
            y = outs["y"].reshape(B, 1)
            return y.astype(np.float32)
        except Exception as e:  # transient axon/device failures
            last_err = e
            global _RUNNER
            _RUNNER = None
    raise last_err
